# revision 2
# baseline (speedup 1.0000x reference)
"""Trainium2 Bass kernel for InnerproductSimilarity — v2.

Differences vs v1 baseline (71918ns):
  - PSUM ring: 4 x [128, 1024] f32 tiles (2 banks each) instead of
    2 x [128, 2048] (4 banks).  With only 2 buffers, at any moment one is
    being filled and ONE drained, so ACT and DVE alternate and each idles
    ~40%.  With 4 slots both engines drain concurrently while the PE fills
    two slots ahead -> both copy engines ~100% busy.  Engine split is
    balanced per chunk by a greedy cost model (~6 ACT : 5 DVE chunks/tile).
  - Support blocks of 512 cols interleaved between PE row halves by bank
    parity (queries duplicated into partitions 64-127), so the two matmuls
    of each 1024-chunk run on different row groups and overlap in the PE.
  - io pool double-buffered: rep k+1 input loads overlap rep k compute;
    support loaded in 2 DMA segments so the first matmuls start earlier.
  - int8 output quantization unchanged (q = round(126.5*simi), host decodes
    out = 0.5 + q*(0.5/126.5); bf16 normalized inputs, scale folded into q).
"""

import numpy as np

N_WAY = 5
K_SHOT = 5
C = 64
HW = 441
M_SUP = K_SHOT * HW      # 2205
Q = 32
N_CORES = 8
QPC = Q // N_CORES       # 4
GI = QPC * HW            # 1764 query positions per core
SF = N_WAY * M_SUP       # 11025 support cols
QSCALE = 126.5

I_TILES = [(128 * t, min(128, GI - 128 * t)) for t in range((GI + 127) // 128)]

# support blocks of 512 cols; even blocks -> row half 0, odd -> half 1
N_BLK = (SF + 511) // 512             # 22 (last block 273)
BLK_W = [min(512, SF - 512 * b) for b in range(N_BLK)]
LO_W = sum(BLK_W[b] for b in range(0, N_BLK, 2))   # 5632
HI_W = sum(BLK_W[b] for b in range(1, N_BLK, 2))   # 5393
S2W = LO_W                                          # hi half padded to this

# chunks of 1024 out cols (2 blocks) per psum slot
N_CHUNK = (N_BLK + 1) // 2            # 11 (last chunk 785)
CHUNK_W = [BLK_W[2 * c] + (BLK_W[2 * c + 1] if 2 * c + 1 < N_BLK else 0)
           for c in range(N_CHUNK)]

# engine cost model (ns) for greedy route balance.  HW-calibrated on the
# axon pool: with alternating engines the per-op overheads hide and ACT
# runs relatively faster than its 1.2 GHz spec (675 ns vs DVE 1300 ns per
# independent-dest [128,1024] PSUM copy), so ACT gets ~58-60% of columns.
COST_ACT = lambda w: (60 + w) / 1.45
COST_DVE = lambda w: (120 + w) / 1.00

# store sub-ranges per i-tile: [(p0, cnt, q, i0)] split at query boundaries
def _store_ranges():
    out = []
    for (g0, im) in I_TILES:
        subs = []
        g = g0
        while g < g0 + im:
            q = g // HW
            g_end = min(g0 + im, (q + 1) * HW)
            subs.append((g - g0, g_end - g, q, g - q * HW))
            g = g_end
        out.append(subs)
    return out


STORES = _store_ranges()

_CACHE = {}

VARIANT = {
    "split_sload": True,
    "io_bufs": 2,
    "psum_bufs": 4,
    "osb_bufs": 3,
    "chunk_banks": 2,   # psum slot width in banks (512 cols each)
    "store_n_groups": 1,  # 1 = one store per (tile, query); 3 = n-groups [2,2,1]
}


def _plan_routes():
    """Greedy per-chunk ACT/DVE assignment balancing engine busy time."""
    busy = {"A": 0.0, "D": 0.0}
    routes = []
    for t in range(len(I_TILES)):
        for c in range(N_CHUNK):
            w = CHUNK_W[c]
            ca, cd = COST_ACT(w), COST_DVE(w)
            if max(busy["A"] + ca, busy["D"]) <= max(busy["A"], busy["D"] + cd):
                busy["A"] += ca
                routes.append("A")
            else:
                busy["D"] += cd
                routes.append("D")
    return routes, busy


def _build(reps=1, mode="full"):
    key = ("nc", reps, mode, tuple(sorted(VARIANT.items())))
    if key in _CACHE:
        return _CACHE[key]
    import concourse.bacc as bacc
    import concourse.mybir as mybir
    import concourse.tile as tile

    nc = bacc.Bacc(
        "TRN2",
        target_bir_lowering=False,
        debug=False,
        enable_asserts=False,
        num_devices=N_CORES,
    )
    f32 = mybir.dt.float32
    bf16 = mybir.dt.bfloat16
    i8 = mybir.dt.int8
    AF = mybir.ActivationFunctionType

    q_in = nc.dram_tensor("q_in", [128, GI], bf16, kind="ExternalInput").ap()
    s_in = nc.dram_tensor("s_in", [128, S2W], bf16, kind="ExternalInput").ap()
    out = nc.dram_tensor(
        "out", [QPC * N_WAY, HW, M_SUP], i8, kind="ExternalOutput"
    ).ap()

    routes, _ = _plan_routes()

    with tile.TileContext(nc) as tc:
        with (
            tc.tile_pool(name="io", bufs=VARIANT["io_bufs"]) as io_pool,
            tc.tile_pool(name="psm", bufs=VARIANT["psum_bufs"], space="PSUM") as psm,
            tc.tile_pool(name="outp", bufs=VARIANT["osb_bufs"]) as out_pool,
        ):
            fixed_osb = None
            for rep in range(reps):
                qn2 = io_pool.tile([128, GI], bf16, tag="qn2")
                sn2 = io_pool.tile([128, S2W], bf16, tag="sn2")
                nc.sync.dma_start(out=qn2, in_=q_in)
                if VARIANT["split_sload"]:
                    mid = 1536
                    nc.scalar.dma_start(out=sn2[:, :mid], in_=s_in[:, :mid])
                    nc.scalar.dma_start(out=sn2[:, mid:], in_=s_in[:, mid:])
                else:
                    nc.scalar.dma_start(out=sn2, in_=s_in)

                if mode == "dmaonly" and fixed_osb is None:
                    fixed_osb = out_pool.tile([128, SF], i8, tag="fixed")
                    nc.vector.memset(fixed_osb, 1)

                ri = 0  # routes index, per-rep
                for t, (g0, im) in enumerate(I_TILES):
                    if mode == "dmaonly":
                        osb = fixed_osb
                        ri += N_CHUNK
                    else:
                        osb = out_pool.tile([128, SF], i8, tag="osb")
                        for c in range(N_CHUNK):
                            cw = CHUNK_W[c]
                            ps = psm.tile([128, 1024], f32, tag="mm")
                            wa = BLK_W[2 * c]
                            off = 512 * c
                            nc.tensor.matmul(
                                ps[:im, 0:wa],
                                qn2[0:64, g0:g0 + im],
                                sn2[0:64, off:off + wa],
                                start=True, stop=True,
                            )
                            if 2 * c + 1 < N_BLK:
                                wb = BLK_W[2 * c + 1]
                                nc.tensor.matmul(
                                    ps[:im, 512:512 + wb],
                                    qn2[64:128, g0:g0 + im],
                                    sn2[64:128, off:off + wb],
                                    start=True, stop=True,
                                )
                            dst = osb[:im, 1024 * c:1024 * c + cw]
                            src = ps[:im, :] if cw == 1024 else ps[:im, 0:cw]
                            if routes[ri] == "A":
                                nc.scalar.activation(dst, src, AF.Copy)
                            else:
                                nc.vector.tensor_scalar_add(dst, src, 0.0)
                            ri += 1
                    if mode == "nodma" and rep != reps - 1:
                        continue
                    dma_eng = nc.sync if t % 2 == 0 else nc.scalar
                    if VARIANT["store_n_groups"] == 1:
                        n_groups = [(0, N_WAY)]
                    else:
                        n_groups = [(0, 2), (2, 2), (4, 1)]
                    for (n0, nk) in n_groups:
                        for (p0, cnt, q, i0) in STORES[t]:
                            dma_eng.dma_start(
                                out=out[
                                    N_WAY * q + n0:N_WAY * q + n0 + nk,
                                    i0:i0 + cnt, :,
                                ].transpose([1, 0, 2]),
                                in_=osb[
                                    p0:p0 + cnt, M_SUP * n0:M_SUP * (n0 + nk)
                                ].rearrange("p (n j) -> p n j", n=nk),
                            )
    nc.compile()
    _CACHE[key] = nc
    return nc


def _get_runner(reps=1, mode="full"):
    key = ("runner", reps, mode, tuple(sorted(VARIANT.items())))
    if key in _CACHE:
        return _CACHE[key]
    import jax
    import jax.numpy as jnp
    from jax.experimental.shard_map import shard_map
    from jax.sharding import Mesh, NamedSharding, PartitionSpec

    import concourse.mybir as mybir
    from concourse import bass2jax

    nc = _build(reps, mode)
    bass2jax.install_neuronx_cc_hook()

    partition_name = nc.partition_id_tensor.name if nc.partition_id_tensor else None
    in_names, out_names, out_avals = [], [], []
    for alloc in nc.m.functions[0].allocations:
        if not isinstance(alloc, mybir.MemoryLocationSet):
            continue
        name = alloc.memorylocations[0].name
        if alloc.kind == "ExternalInput":
            if name == partition_name:
                continue
            in_names.append(name)
        elif alloc.kind == "ExternalOutput":
            out_names.append(name)
            out_avals.append(
                jax.core.ShapedArray(
                    tuple(alloc.tensor_shape), mybir.dt.np(alloc.dtype)
                )
            )
    n_params = len(in_names)
    n_outs = len(out_names)
    all_in_names = tuple(in_names) + tuple(out_names)
    if partition_name is not None:
        all_in_names = all_in_names + (partition_name,)

    def _body(*args):
        operands = list(args)
        if partition_name is not None:
            operands.append(bass2jax.partition_id_tensor())
        outs = bass2jax._bass_exec_p.bind(
            *operands,
            out_avals=tuple(out_avals),
            in_names=all_in_names,
            out_names=tuple(out_names),
            lowering_input_output_aliases=(),
            sim_require_finite=True,
            sim_require_nnan=True,
            nc=nc,
        )
        return tuple(outs)

    devices = jax.devices()[:N_CORES]
    assert len(devices) == N_CORES, f"need {N_CORES} cores, have {len(jax.devices())}"
    mesh = Mesh(np.asarray(devices), ("core",))
    in_specs = (PartitionSpec("core"),) * (n_params + n_outs)
    out_specs = (PartitionSpec("core"),) * n_outs
    donate = tuple(range(n_params, n_params + n_outs))
    sharded = jax.jit(
        shard_map(
            _body, mesh=mesh, in_specs=in_specs, out_specs=out_specs, check_rep=False
        ),
        donate_argnums=donate,
        keep_unused=True,
    )
    shard = NamedSharding(mesh, PartitionSpec("core"))
    zero_shapes = [(N_CORES * a.shape[0], *a.shape[1:]) for a in out_avals]
    zeros_fn = jax.jit(
        lambda: tuple(
            jnp.zeros(s, a.dtype) for s, a in zip(zero_shapes, out_avals)
        ),
        out_shardings=(shard,) * n_outs,
    )
    _CACHE[key] = (sharded, zeros_fn, in_names, out_names, shard)
    return _CACHE[key]


def _prep_inputs(support_xf, query_xf):
    """Host-side layout prep: channel-major normalized bf16; queries
    duplicated across row halves; support 512-col blocks interleaved by
    parity between row halves."""
    import ml_dtypes

    s_cm = np.ascontiguousarray(
        support_xf.reshape(N_WAY, K_SHOT, C, HW)
        .transpose(2, 0, 1, 3)
        .reshape(C, SF)
    ).astype(np.float32, copy=False)
    s_cm = s_cm / np.linalg.norm(s_cm, axis=0, keepdims=True)
    blocks = [s_cm[:, 512 * b:512 * b + BLK_W[b]] for b in range(N_BLK)]
    lo = np.concatenate([blocks[b] for b in range(0, N_BLK, 2)], axis=1)
    hi = np.concatenate(
        [blocks[b] for b in range(1, N_BLK, 2)]
        + [np.ones((C, S2W - HI_W), np.float32)],
        axis=1,
    )
    s2 = np.concatenate([lo, hi], axis=0)  # [128, S2W]
    assert s2.shape == (128, S2W)

    q_all = query_xf.reshape(Q, C, HW)
    q_parts = []
    for k in range(N_CORES):
        q_cm = q_all[k * QPC:(k + 1) * QPC].transpose(1, 0, 2).reshape(C, GI)
        q_cm = q_cm * (QSCALE / np.linalg.norm(q_cm, axis=0, keepdims=True))
        q_parts.append(np.concatenate([q_cm, q_cm], axis=0))  # [128, GI]
    q_cat = np.concatenate(q_parts, axis=0).astype(ml_dtypes.bfloat16)
    s_cat = np.concatenate([s2] * N_CORES, axis=0).astype(ml_dtypes.bfloat16)
    return {
        "q_in": np.ascontiguousarray(q_cat),
        "s_in": np.ascontiguousarray(s_cat),
    }


def kernel(support_xf, support_y, query_xf, query_y):
    import jax

    assert support_xf.shape == (1, N_WAY * K_SHOT, C, 21, 21)
    assert query_xf.shape == (1, Q, C, 21, 21)

    sharded, zeros_fn, in_names, out_names, shard = _get_runner()
    cat = _prep_inputs(support_xf, query_xf)
    args = [jax.device_put(cat[n], shard) for n in in_names]
    outs = sharded(*args, *zeros_fn())
    out_q = np.asarray(outs[0])  # [8*20, 441, 2205] int8, core-major
    out = out_q.astype(np.float32)
    out *= 0.5 / QSCALE
    out += 0.5
    return out.reshape(1, Q, N_WAY, HW, M_SUP)


# revision 4
# speedup vs baseline: 1.1475x; 1.1475x over previous
"""Trainium2 Bass kernel for InnerproductSimilarity — v2.

Differences vs v1 baseline (71918ns):
  - PSUM ring: 4 x [128, 1024] f32 tiles (2 banks each) instead of
    2 x [128, 2048] (4 banks).  With only 2 buffers, at any moment one is
    being filled and ONE drained, so ACT and DVE alternate and each idles
    ~40%.  With 4 slots both engines drain concurrently while the PE fills
    two slots ahead -> both copy engines ~100% busy.  Engine split is
    balanced per chunk by a greedy cost model (~6 ACT : 5 DVE chunks/tile).
  - Support blocks of 512 cols interleaved between PE row halves by bank
    parity (queries duplicated into partitions 64-127), so the two matmuls
    of each 1024-chunk run on different row groups and overlap in the PE.
  - io pool double-buffered: rep k+1 input loads overlap rep k compute;
    support loaded in 2 DMA segments so the first matmuls start earlier.
  - int8 output quantization unchanged (q = round(126.5*simi), host decodes
    out = 0.5 + q*(0.5/126.5); bf16 normalized inputs, scale folded into q).
"""

import numpy as np

N_WAY = 5
K_SHOT = 5
C = 64
HW = 441
M_SUP = K_SHOT * HW      # 2205
Q = 32
N_CORES = 8
QPC = Q // N_CORES       # 4
GI = QPC * HW            # 1764 query positions per core
SF = N_WAY * M_SUP       # 11025 support cols
QSCALE = 126.5

I_TILES = [(128 * t, min(128, GI - 128 * t)) for t in range((GI + 127) // 128)]

# support blocks of 512 cols; even blocks -> row half 0, odd -> half 1
N_BLK = (SF + 511) // 512             # 22 (last block 273)
BLK_W = [min(512, SF - 512 * b) for b in range(N_BLK)]
LO_W = sum(BLK_W[b] for b in range(0, N_BLK, 2))   # 5632
HI_W = sum(BLK_W[b] for b in range(1, N_BLK, 2))   # 5393
S2W = LO_W                                          # hi half padded to this

# chunks of 1024 out cols (2 blocks) per psum slot
N_CHUNK = (N_BLK + 1) // 2            # 11 (last chunk 785)
CHUNK_W = [BLK_W[2 * c] + (BLK_W[2 * c + 1] if 2 * c + 1 < N_BLK else 0)
           for c in range(N_CHUNK)]

# engine cost model (ns) for greedy route balance.  HW-calibrated on the
# axon pool: with alternating engines the per-op overheads hide and ACT
# runs relatively faster than its 1.2 GHz spec (675 ns vs DVE 1300 ns per
# independent-dest [128,1024] PSUM copy), so ACT gets ~58-60% of columns.
COST_ACT = lambda w: (60 + w) / 1.45
COST_DVE = lambda w: (120 + w) / 1.00

# store sub-ranges per i-tile: [(p0, cnt, q, i0)] split at query boundaries
def _store_ranges():
    out = []
    for (g0, im) in I_TILES:
        subs = []
        g = g0
        while g < g0 + im:
            q = g // HW
            g_end = min(g0 + im, (q + 1) * HW)
            subs.append((g - g0, g_end - g, q, g - q * HW))
            g = g_end
        out.append(subs)
    return out


STORES = _store_ranges()

_CACHE = {}

VARIANT = {
    "split_sload": True,
    "io_bufs": 2,
    "psum_bufs": 4,
    "osb_bufs": 3,
    "chunk_banks": 2,   # psum slot width in banks (512 cols each)
    "store_n_groups": 1,  # 1 = one store per (tile, query); 3 = n-groups [2,2,1]
    "alt_routes": True,   # strict ACT/DVE alternation (HW: ~2.6x vs 60:40)
}


def _plan_routes():
    """Per-chunk ACT/DVE assignment.

    HW A/B (pipe4 vs pipe4_60 microbench): STRICT alternation sustains
    ~192 ns per 1024-col chunk vs ~510 ns for a 60:40 pattern with
    back-to-back same-engine ops — alternation lets each engine's post-op
    ack/drain overhead hide under the other engine's op.  So default is
    strict ADAD...; the greedy balance is kept as a fallback variant.
    """
    if VARIANT.get("alt_routes", True):
        n = len(I_TILES) * N_CHUNK
        routes = ["A" if i % 2 == 0 else "D" for i in range(n)]
        return routes, {}
    busy = {"A": 0.0, "D": 0.0}
    routes = []
    for t in range(len(I_TILES)):
        for c in range(N_CHUNK):
            w = CHUNK_W[c]
            ca, cd = COST_ACT(w), COST_DVE(w)
            if max(busy["A"] + ca, busy["D"]) <= max(busy["A"], busy["D"] + cd):
                busy["A"] += ca
                routes.append("A")
            else:
                busy["D"] += cd
                routes.append("D")
    return routes, busy


def _build(reps=1, mode="full"):
    key = ("nc", reps, mode, tuple(sorted(VARIANT.items())))
    if key in _CACHE:
        return _CACHE[key]
    import concourse.bacc as bacc
    import concourse.mybir as mybir
    import concourse.tile as tile

    nc = bacc.Bacc(
        "TRN2",
        target_bir_lowering=False,
        debug=False,
        enable_asserts=False,
        num_devices=N_CORES,
    )
    f32 = mybir.dt.float32
    bf16 = mybir.dt.bfloat16
    i8 = mybir.dt.int8
    AF = mybir.ActivationFunctionType

    q_in = nc.dram_tensor("q_in", [128, GI], bf16, kind="ExternalInput").ap()
    s_in = nc.dram_tensor("s_in", [128, S2W], bf16, kind="ExternalInput").ap()
    out = nc.dram_tensor(
        "out", [QPC * N_WAY, HW, M_SUP], i8, kind="ExternalOutput"
    ).ap()

    routes, _ = _plan_routes()

    with tile.TileContext(nc) as tc:
        with (
            tc.tile_pool(name="io", bufs=VARIANT["io_bufs"]) as io_pool,
            tc.tile_pool(name="psm", bufs=VARIANT["psum_bufs"], space="PSUM") as psm,
            tc.tile_pool(name="outp", bufs=VARIANT["osb_bufs"]) as out_pool,
        ):
            fixed_osb = None
            for rep in range(reps):
                qn2 = io_pool.tile([128, GI], bf16, tag="qn2")
                sn2 = io_pool.tile([128, S2W], bf16, tag="sn2")
                nc.sync.dma_start(out=qn2, in_=q_in)
                if VARIANT["split_sload"]:
                    mid = 1536
                    nc.scalar.dma_start(out=sn2[:, :mid], in_=s_in[:, :mid])
                    nc.scalar.dma_start(out=sn2[:, mid:], in_=s_in[:, mid:])
                else:
                    nc.scalar.dma_start(out=sn2, in_=s_in)

                if mode == "dmaonly" and fixed_osb is None:
                    fixed_osb = out_pool.tile([128, SF], i8, tag="fixed")
                    nc.vector.memset(fixed_osb, 1)

                ri = 0  # routes index, per-rep
                for t, (g0, im) in enumerate(I_TILES):
                    if mode == "dmaonly":
                        osb = fixed_osb
                        ri += N_CHUNK
                    else:
                        osb = out_pool.tile([128, SF], i8, tag="osb")
                        for c in range(N_CHUNK):
                            cw = CHUNK_W[c]
                            ps = psm.tile([128, 1024], f32, tag="mm")
                            wa = BLK_W[2 * c]
                            off = 512 * c
                            nc.tensor.matmul(
                                ps[:im, 0:wa],
                                qn2[0:64, g0:g0 + im],
                                sn2[0:64, off:off + wa],
                                start=True, stop=True,
                            )
                            if 2 * c + 1 < N_BLK:
                                wb = BLK_W[2 * c + 1]
                                nc.tensor.matmul(
                                    ps[:im, 512:512 + wb],
                                    qn2[64:128, g0:g0 + im],
                                    sn2[64:128, off:off + wb],
                                    start=True, stop=True,
                                )
                            dst = osb[:im, 1024 * c:1024 * c + cw]
                            src = ps[:im, :] if cw == 1024 else ps[:im, 0:cw]
                            if routes[ri] == "A":
                                nc.scalar.activation(dst, src, AF.Copy)
                            else:
                                nc.vector.tensor_copy(dst, src)
                            ri += 1
                    if mode == "nodma" and rep != reps - 1:
                        continue
                    dma_eng = nc.sync if t % 2 == 0 else nc.scalar
                    if VARIANT["store_n_groups"] == 1:
                        n_groups = [(0, N_WAY)]
                    else:
                        n_groups = [(0, 2), (2, 2), (4, 1)]
                    for (n0, nk) in n_groups:
                        for (p0, cnt, q, i0) in STORES[t]:
                            dma_eng.dma_start(
                                out=out[
                                    N_WAY * q + n0:N_WAY * q + n0 + nk,
                                    i0:i0 + cnt, :,
                                ].transpose([1, 0, 2]),
                                in_=osb[
                                    p0:p0 + cnt, M_SUP * n0:M_SUP * (n0 + nk)
                                ].rearrange("p (n j) -> p n j", n=nk),
                            )
    nc.compile()
    _CACHE[key] = nc
    return nc


def _get_runner(reps=1, mode="full"):
    key = ("runner", reps, mode, tuple(sorted(VARIANT.items())))
    if key in _CACHE:
        return _CACHE[key]
    import jax
    import jax.numpy as jnp
    from jax.experimental.shard_map import shard_map
    from jax.sharding import Mesh, NamedSharding, PartitionSpec

    import concourse.mybir as mybir
    from concourse import bass2jax

    nc = _build(reps, mode)
    bass2jax.install_neuronx_cc_hook()

    partition_name = nc.partition_id_tensor.name if nc.partition_id_tensor else None
    in_names, out_names, out_avals = [], [], []
    for alloc in nc.m.functions[0].allocations:
        if not isinstance(alloc, mybir.MemoryLocationSet):
            continue
        name = alloc.memorylocations[0].name
        if alloc.kind == "ExternalInput":
            if name == partition_name:
                continue
            in_names.append(name)
        elif alloc.kind == "ExternalOutput":
            out_names.append(name)
            out_avals.append(
                jax.core.ShapedArray(
                    tuple(alloc.tensor_shape), mybir.dt.np(alloc.dtype)
                )
            )
    n_params = len(in_names)
    n_outs = len(out_names)
    all_in_names = tuple(in_names) + tuple(out_names)
    if partition_name is not None:
        all_in_names = all_in_names + (partition_name,)

    def _body(*args):
        operands = list(args)
        if partition_name is not None:
            operands.append(bass2jax.partition_id_tensor())
        outs = bass2jax._bass_exec_p.bind(
            *operands,
            out_avals=tuple(out_avals),
            in_names=all_in_names,
            out_names=tuple(out_names),
            lowering_input_output_aliases=(),
            sim_require_finite=True,
            sim_require_nnan=True,
            nc=nc,
        )
        return tuple(outs)

    devices = jax.devices()[:N_CORES]
    assert len(devices) == N_CORES, f"need {N_CORES} cores, have {len(jax.devices())}"
    mesh = Mesh(np.asarray(devices), ("core",))
    in_specs = (PartitionSpec("core"),) * (n_params + n_outs)
    out_specs = (PartitionSpec("core"),) * n_outs
    donate = tuple(range(n_params, n_params + n_outs))
    sharded = jax.jit(
        shard_map(
            _body, mesh=mesh, in_specs=in_specs, out_specs=out_specs, check_rep=False
        ),
        donate_argnums=donate,
        keep_unused=True,
    )
    shard = NamedSharding(mesh, PartitionSpec("core"))
    zero_shapes = [(N_CORES * a.shape[0], *a.shape[1:]) for a in out_avals]
    zeros_fn = jax.jit(
        lambda: tuple(
            jnp.zeros(s, a.dtype) for s, a in zip(zero_shapes, out_avals)
        ),
        out_shardings=(shard,) * n_outs,
    )
    _CACHE[key] = (sharded, zeros_fn, in_names, out_names, shard)
    return _CACHE[key]


def _prep_inputs(support_xf, query_xf):
    """Host-side layout prep: channel-major normalized bf16; queries
    duplicated across row halves; support 512-col blocks interleaved by
    parity between row halves."""
    import ml_dtypes

    s_cm = np.ascontiguousarray(
        support_xf.reshape(N_WAY, K_SHOT, C, HW)
        .transpose(2, 0, 1, 3)
        .reshape(C, SF)
    ).astype(np.float32, copy=False)
    s_cm = s_cm / np.linalg.norm(s_cm, axis=0, keepdims=True)
    blocks = [s_cm[:, 512 * b:512 * b + BLK_W[b]] for b in range(N_BLK)]
    lo = np.concatenate([blocks[b] for b in range(0, N_BLK, 2)], axis=1)
    hi = np.concatenate(
        [blocks[b] for b in range(1, N_BLK, 2)]
        + [np.ones((C, S2W - HI_W), np.float32)],
        axis=1,
    )
    s2 = np.concatenate([lo, hi], axis=0)  # [128, S2W]
    assert s2.shape == (128, S2W)

    q_all = query_xf.reshape(Q, C, HW)
    q_parts = []
    for k in range(N_CORES):
        q_cm = q_all[k * QPC:(k + 1) * QPC].transpose(1, 0, 2).reshape(C, GI)
        q_cm = q_cm * (QSCALE / np.linalg.norm(q_cm, axis=0, keepdims=True))
        q_parts.append(np.concatenate([q_cm, q_cm], axis=0))  # [128, GI]
    q_cat = np.concatenate(q_parts, axis=0).astype(ml_dtypes.bfloat16)
    s_cat = np.concatenate([s2] * N_CORES, axis=0).astype(ml_dtypes.bfloat16)
    return {
        "q_in": np.ascontiguousarray(q_cat),
        "s_in": np.ascontiguousarray(s_cat),
    }


def kernel(support_xf, support_y, query_xf, query_y):
    import jax

    assert support_xf.shape == (1, N_WAY * K_SHOT, C, 21, 21)
    assert query_xf.shape == (1, Q, C, 21, 21)

    sharded, zeros_fn, in_names, out_names, shard = _get_runner()
    cat = _prep_inputs(support_xf, query_xf)
    args = [jax.device_put(cat[n], shard) for n in in_names]
    outs = sharded(*args, *zeros_fn())
    out_q = np.asarray(outs[0])  # [8*20, 441, 2205] int8, core-major
    out = out_q.astype(np.float32)
    out *= 0.5 / QSCALE
    out += 0.5
    return out.reshape(1, Q, N_WAY, HW, M_SUP)
